# revision 30
# baseline (speedup 1.0000x reference)
"""ArcFace loss (B=1024, D=256, C=50000) distributed over 8 TRN2 NeuronCores.

Strategy: shard the class dimension (6250 classes/core, zero-padded to 6272).
Host passes the weight shard pre-transposed ([D, CP], bf16, zero-padded) plus
raw-transposed embeddings.  Per core, on device:
  - |w_c| via TensorE self-matmul (gram diagonal, extracted with an identity
    mask multiply + free-axis reduce on VectorE),
  - all 1/sqrt computed as exp(-0.5*ln(x)) on ScalarE so the whole kernel uses
    a single activation table set (Ln/Exp/Square) - no table-swap stalls,
  - inv-norm row broadcast across partitions (GpSimd) and applied to wT with
    one VectorE multiply (bf16 2x mode),
  - cos = eT.T @ wT_norm on TensorE (bf16), exp on ScalarE with per-partition
    scale 30/|e_b| (folding the embedding norm into the activation scale) and
    free-axis accumulation producing the partial sum-exp,
  - the margin (target-class) term on a [128]-row slice per core from
    host-gathered target weight rows, using
    cos(theta+m) = cos*cos(m) - sqrt(1-cos^2)*sin(m); computed up front so its
    AllGather overlaps the main loop,
  - AllReduce(partial sum-exp) at the end, then log-sum-exp -> mean on device.
The c dimension is processed in groups (4+8+12+12+13 tiles of 128) so the
norm/scale pipeline of group q overlaps the matmul/exp of group q-1, with a
small first group to minimize the serial prefix.
"""

import os
import sys

sys.path.insert(0, "/opt/trn_rl_repo")

import numpy as np
import ml_dtypes

B, D, C = 1024, 256, 50000
NCORES = 8
CS = C // NCORES          # 6250 classes per core
CT = 49                   # 128-class tiles per core
CP = CT * 128             # 6272 (padded)
PADS = CP - CS            # 22 zero-pad classes per core
SCALE = 30.0
MARGIN = 0.5
COSM = float(np.cos(MARGIN))
SINM = float(np.sin(MARGIN))
EPS = 1e-7

# c-tile pipeline groups (in 128-class tiles): small first group -> short prefix
QT = [(0, 5), (5, 8), (13, 12), (25, 12), (37, 12)]
# main-loop c chunks, aligned to group boundaries, <=1536 (3 PSUM banks)
CCH = [(0, 640), (640, 1024), (1664, 1536), (3200, 1536), (4736, 1536)]

_cached_nc = None


def _build(variant="full", niter=1):
    # variant: comma-set of stage-skip flags for benchmarking attribution.
    #   nocc    - skip collectives (use local data instead)
    #   nomain  - skip main matmul+exp loop
    #   nonorm  - skip w-norm pipeline (use raw wT in main loop)
    #   nobcast - replace partition_broadcast with a memset
    vset = set(variant.split(",")) if variant else set()
    from concourse import bacc, tile, mybir

    # Force every ScalarE activation into the one table set that holds all the
    # functions this kernel uses (Ln/Exp/Square/Copy) so the whole NEFF does a
    # single ACT_TABLE_LOAD.  The chooser picks the first set containing each
    # func; hiding these funcs from the other sets (positions preserved, so
    # emitted act_func_set_ids stay valid) redirects it to the combined set.
    import concourse.bacc as _bacc_mod
    from concourse import hw_specs as _hw_specs
    _KEEP = "natural_log_exp_and_others"
    _HIDE = {"Exp", "Ln", "Square", "Copy"}
    if not getattr(_bacc_mod, "_act_tables_patched", False):
        _orig_gat = _hw_specs.get_activation_tables

        def _patched_gat(arch, *a, **kw):
            tabs = _orig_gat(arch, *a, **kw)
            keep = tabs.get(_KEEP)
            if not keep or not _HIDE.issubset({f.name for f in keep}):
                return tabs  # unexpected table layout: leave untouched
            return {
                name: (funcs if name == _KEEP
                       else {f for f in funcs if f.name not in _HIDE})
                for name, funcs in tabs.items()
            }

        try:
            _bacc_mod.get_activation_tables = _patched_gat
            _bacc_mod._act_tables_patched = True
        except Exception:
            pass

    f32 = mybir.dt.float32
    bf16 = mybir.dt.bfloat16
    fp8 = mybir.dt.float8e4
    ALU = mybir.AluOpType
    ACT = mybir.ActivationFunctionType
    AX = mybir.AxisListType

    nc = bacc.Bacc("TRN2", target_bir_lowering=False, debug=False,
                   num_devices=NCORES)

    wtd = nc.dram_tensor("wt", [D, CP], fp8, kind="ExternalInput")
    eTd = nc.dram_tensor("eT", [D, B], fp8, kind="ExternalInput")
    ed = nc.dram_tensor("e", [B, D], f32, kind="ExternalInput")
    twd = nc.dram_tensor("tw", [128, D], f32, kind="ExternalInput")
    esd = nc.dram_tensor("es", [128, D], f32, kind="ExternalInput")
    idd = nc.dram_tensor("idm", [128, 128], f32, kind="ExternalInput")
    outd = nc.dram_tensor("out", [1, 1], f32, kind="ExternalOutput")

    with tile.TileContext(nc) as tc:
        with (
            tc.tile_pool(name="sb", bufs=1) as sb,
            tc.tile_pool(name="ps", bufs=2, space="PSUM") as ps,
            tc.tile_pool(name="gps", bufs=2, space="PSUM") as gps,
            tc.tile_pool(name="dr", bufs=1, space="DRAM") as dr,
        ):
            # ---------------- persistent SBUF tensors ----------------
            wTr = sb.tile([128, 2, CP], fp8)      # raw transposed weights
            wTn = sb.tile([128, 2, CP], fp8)      # normalized
            bcast = sb.tile([128, CP], f32)       # broadcast inv |w_c| row
            eTs = sb.tile([128, 2, B], fp8)
            e_nat = sb.tile([128, 8, D], f32)
            tw_s = sb.tile([128, D], f32)
            es_s = sb.tile([128, D], f32)
            idm = sb.tile([128, 128], f32)
            wt_ap = wtd.ap().rearrange("(k p) c -> p k c", p=128)

            sq_scr = sb.tile([128, 8 * D], f32)   # elementwise-square scratch
            eps_t = sb.tile([128, 1], f32)        # tiny Ln bias: keeps ln(0) finite
            ln30_t = sb.tile([128, 1], f32)       # ln(30) bias for 30/sqrt(x)
            padc_t = sb.tile([128, 1], f32)       # -(pad count) Ln bias
            esq = sb.tile([128, 8], f32)
            lesq = sb.tile([128, 8], f32)
            inv_e30 = sb.tile([128, 8], f32)
            wsq = sb.tile([128, CT], f32)
            lwsq = sb.tile([128, CT], f32)
            inv_wn = sb.tile([128, CT], f32)
            dscr = sb.tile([128, 13, 128], f32)
            rtmp = dr.tile([CP], f32)
            rtmp_pt = rtmp[:].rearrange("(t p) -> p t", p=128)
            rtmp_row = rtmp[:].rearrange("(o c) -> o c", o=1)
            NCC = len(CCH)
            sacc = sb.tile([128, 8 * NCC], f32)
            escr = sb.tile([128, 1536], bf16)
            Sp = sb.tile([128, 8], f32)
            # target-path tiles
            tesq = sb.tile([128, 1], f32)
            tdot = sb.tile([128, 1], f32)
            twsq = sb.tile([128, 1], f32)
            tln = sb.tile([128, 2], f32)
            tinv = sb.tile([128, 2], f32)
            prodd = sb.tile([128, D], f32)
            prod2 = sb.tile([128, D], f32)
            prod3 = sb.tile([128, D], f32)
            ct0 = sb.tile([128, 1], f32)
            ctc = sb.tile([128, 1], f32)
            cos2 = sb.tile([128, 1], f32)
            omc = sb.tile([128, 1], f32)
            lnomc = sb.tile([128, 1], f32)
            sin_t = sb.tile([128, 1], f32)
            ca = sb.tile([128, 1], f32)
            cb = sb.tile([128, 1], f32)
            cosm = sb.tile([128, 1], f32)
            pair = sb.tile([128, 2], f32)
            expts = sb.tile([128, 2], f32)
            ctpay = sb.tile([128, 2], f32)
            # collective buffers
            s_in = dr.tile([128, 8], f32)
            s_out = dr.tile([128, 8], f32)
            c_in = dr.tile([128, 2], f32)
            c_out = dr.tile([B, 2], f32)
            S_ar = sb.tile([128, 8], f32)
            ctg = sb.tile([128, 8, 2], f32)
            S1 = sb.tile([128, 8], f32)
            S2 = sb.tile([128, 8], f32)
            lse = sb.tile([128, 8], f32)
            nll = sb.tile([128, 8], f32)
            rsum = sb.tile([128, 1], f32)
            ones = sb.tile([128, 1], f32)
            res = sb.tile([1, 1], f32)
            wmm = wTr if "nonorm" in vset else wTn
            grp = [list(range(NCORES))]

            for _it in range(niter):
                # ---- input DMAs: first c-group's weights first (critical path)
                (g0, gn) = QT[0]
                nc.sync.dma_start(out=wTr[:, :, g0 * 128:(g0 + gn) * 128],
                                  in_=wt_ap[:, :, g0 * 128:(g0 + gn) * 128])
                nc.sync.dma_start(out=idm[:], in_=idd.ap())
                nc.sync.dma_start(out=e_nat[:], in_=ed.ap().rearrange("(t p) d -> p t d", p=128))
                nc.sync.dma_start(out=eTs[:], in_=eTd.ap().rearrange("(k p) b -> p k b", p=128))
                nc.sync.dma_start(out=tw_s[:], in_=twd.ap())
                nc.sync.dma_start(out=es_s[:], in_=esd.ap())
                for (t0, nt) in QT[1:]:
                    nc.sync.dma_start(out=wTr[:, :, t0 * 128:(t0 + nt) * 128],
                                      in_=wt_ap[:, :, t0 * 128:(t0 + nt) * 128])

                nc.vector.memset(eps_t[:], 1e-20)
                nc.vector.memset(ln30_t[:], float(np.log(SCALE)))
                nc.vector.memset(padc_t[:], -float(PADS * NCORES))
                nc.vector.memset(ones[:], 1.0 / B)

                # ---- weight norms (gram diag) + normalize, per c-group.
                # Group 0 first (it gates the first main matmul); the e-norm
                # and target paths slot in behind it, then the later groups.
                def _norm_group(t0, nt):
                    for t in range(t0, t0 + nt):
                        g = gps.tile([128, 128], f32, name=f"g{t}", tag="g")
                        nc.tensor.matmul(
                            g[:], lhsT=wTr[:, :, t * 128:(t + 1) * 128],
                            rhs=wTr[:, :, t * 128:(t + 1) * 128],
                            perf_mode=mybir.MatmulPerfMode.DoubleRow,
                            start=True, stop=True)
                        nc.vector.tensor_mul(dscr[:, t - t0], g[:], idm[:])
                    nc.vector.tensor_reduce(out=wsq[:, t0:t0 + nt],
                                            in_=dscr[:, :nt], axis=AX.X, op=ALU.add)
                    nc.scalar.activation(lwsq[:, t0:t0 + nt], wsq[:, t0:t0 + nt],
                                         ACT.Ln, bias=eps_t[:])
                    nc.scalar.activation(inv_wn[:, t0:t0 + nt], lwsq[:, t0:t0 + nt],
                                         ACT.Exp, scale=-0.5)
                    # reorient to a row in DRAM, then DMA the row back
                    # broadcast across all 128 partitions
                    nc.sync.dma_start(out=rtmp_pt[:, t0:t0 + nt],
                                      in_=inv_wn[:, t0:t0 + nt])
                    nc.sync.dma_start(
                        out=bcast[:, t0 * 128:(t0 + nt) * 128],
                        in_=rtmp_row[:, t0 * 128:(t0 + nt) * 128].broadcast_to(
                            (128, nt * 128)))
                    for k in range(2):
                        nc.vector.tensor_mul(wTn[:, k, t0 * 128:(t0 + nt) * 128],
                                             wTr[:, k, t0 * 128:(t0 + nt) * 128],
                                             bcast[:, t0 * 128:(t0 + nt) * 128])

                ngroups = QT if "nonorm" not in vset else []
                if ngroups:
                    _norm_group(*ngroups[0])
                # ---- embedding norms: 30/|e_b| = exp(-0.5 ln(esq) + ln 30)
                for t in range(8):
                    nc.scalar.activation(sq_scr[:, :D], e_nat[:, t], ACT.Square,
                                         accum_out=esq[:, t:t + 1])
                nc.scalar.activation(lesq[:], esq[:], ACT.Ln, bias=eps_t[:])
                nc.scalar.activation(inv_e30[:], lesq[:], ACT.Exp, scale=-0.5,
                                     bias=ln30_t[:])

                # ---- target/margin path first: its AllGather overlaps the rest
                nc.gpsimd.tensor_mul(prodd[:], es_s[:], es_s[:])
                nc.vector.tensor_reduce(out=tesq[:], in_=prodd[:], axis=AX.X, op=ALU.add)
                nc.gpsimd.tensor_mul(prod2[:], tw_s[:], tw_s[:])
                nc.vector.tensor_reduce(out=twsq[:], in_=prod2[:], axis=AX.X, op=ALU.add)
                nc.gpsimd.tensor_mul(prod3[:], es_s[:], tw_s[:])
                nc.vector.tensor_reduce(out=tdot[:], in_=prod3[:], axis=AX.X, op=ALU.add)
                # 1/sqrt via exp(-ln/2): single activation table set
                nc.vector.tensor_copy(pair[:, 0:1], tesq[:])
                nc.vector.tensor_copy(pair[:, 1:2], twsq[:])
                nc.scalar.activation(tln[:], pair[:], ACT.Ln, bias=eps_t[:])
                nc.scalar.activation(tinv[:], tln[:], ACT.Exp, scale=-0.5)
                nc.vector.tensor_mul(ct0[:], tdot[:], tinv[:, 0:1])
                nc.vector.tensor_mul(ctc[:], ct0[:], tinv[:, 1:2])
                nc.vector.tensor_scalar_min(ctc[:], ctc[:], 1.0 - EPS)
                nc.vector.tensor_scalar_max(ctc[:], ctc[:], -1.0 + EPS)
                nc.scalar.activation(cos2[:], ctc[:], ACT.Square)
                nc.vector.tensor_scalar(out=omc[:], in0=cos2[:], scalar1=-1.0,
                                        scalar2=1.0, op0=ALU.mult, op1=ALU.add)
                # sqrt(x) = exp(+ln/2)
                nc.scalar.activation(lnomc[:], omc[:], ACT.Ln, bias=eps_t[:])
                nc.scalar.activation(sin_t[:], lnomc[:], ACT.Exp, scale=0.5)
                nc.vector.tensor_scalar_mul(ca[:], ctc[:], COSM)
                nc.vector.tensor_scalar_mul(cb[:], sin_t[:], SINM)
                nc.vector.tensor_sub(cosm[:], ca[:], cb[:])
                nc.vector.tensor_copy(pair[:, 0:1], ctc[:])
                nc.vector.tensor_copy(pair[:, 1:2], cosm[:])
                nc.scalar.activation(expts[:], pair[:], ACT.Exp, scale=SCALE)
                nc.vector.tensor_sub(ctpay[:, 0:1], expts[:, 1:2], expts[:, 0:1])
                nc.vector.tensor_scalar_mul(ctpay[:, 1:2], cosm[:], SCALE)
                nc.sync.dma_start(out=c_in[:], in_=ctpay[:])
                if "nocc" not in vset:
                    nc.gpsimd.collective_compute(
                        "AllGather", ALU.bypass, replica_groups=grp,
                        ins=[c_in.opt()], outs=[c_out.opt()])
                    nc.sync.dma_start(
                        out=ctg[:], in_=c_out[:].rearrange("(t p) r -> p t r", p=128))
                else:
                    nc.vector.memset(ctg[:], 0.5)

                for (t0, nt) in ngroups[1:]:
                    _norm_group(t0, nt)

                # ---- main matmul + exp + partial sumexp (c-major) ----
                for ci, (off, cs) in enumerate(CCH if "nomain" not in vset else []):
                    for bt in range(8):
                        pt = ps.tile([128, 1536], f32, name=f"pt{ci}_{bt}", tag="pt")
                        for j in range((cs + 511) // 512):
                            n0 = j * 512
                            n1 = min(cs, n0 + 512)
                            nc.tensor.matmul(
                                pt[:, n0:n1],
                                lhsT=eTs[:, :, bt * 128:(bt + 1) * 128],
                                rhs=wmm[:, :, off + n0:off + n1],
                                perf_mode=mybir.MatmulPerfMode.DoubleRow,
                                start=True, stop=True)
                        nc.scalar.activation(
                            escr[:, :cs], pt[:, :cs], ACT.Exp,
                            scale=inv_e30[:, bt:bt + 1],
                            accum_out=sacc[:, bt * NCC + ci:bt * NCC + ci + 1])

                if "nomain" in vset:
                    nc.vector.memset(sacc[:], 1.0)
                nc.vector.tensor_reduce(
                    out=Sp[:], in_=sacc[:].rearrange("p (t c) -> p t c", c=NCC),
                    axis=AX.X, op=ALU.add)
                nc.sync.dma_start(out=s_in[:], in_=Sp[:])
                if "nocc" not in vset:
                    nc.gpsimd.collective_compute(
                        "AllReduce", ALU.add, replica_groups=grp,
                        ins=[s_in.opt()], outs=[s_out.opt()])
                    nc.sync.dma_start(out=S_ar[:], in_=s_out[:])
                else:
                    nc.sync.dma_start(out=S_ar[:], in_=s_in[:])

                nc.vector.tensor_add(S1[:], S_ar[:], ctg[:, :, 0])
                # Ln(S - pads): the zero-pad classes contribute exp(0)=1 each
                nc.scalar.activation(lse[:], S1[:], ACT.Ln, bias=padc_t[:])
                nc.vector.tensor_sub(nll[:], lse[:], ctg[:, :, 1])
                nc.vector.tensor_reduce(out=rsum[:], in_=nll[:], axis=AX.X, op=ALU.add)
                psf = gps.tile([128, 128], f32, name="psf", tag="g")
                nc.tensor.matmul(psf[0:1, 0:1], lhsT=ones[:], rhs=rsum[:],
                                 start=True, stop=True)
                nc.scalar.copy(res[:], psf[0:1, 0:1])
                nc.sync.dma_start(out=outd.ap(), in_=res[:])

    nc.compile()
    return nc


def _prep_inputs(embeddings, labels, weight):
    emb = np.ascontiguousarray(np.asarray(embeddings), dtype=np.float32)
    lab = np.asarray(labels).astype(np.int64)
    w = np.ascontiguousarray(np.asarray(weight), dtype=np.float32)

    f8 = ml_dtypes.float8_e4m3
    eT_bf = np.ascontiguousarray(emb.T).astype(f8)
    # per-core transposed, zero-padded weight shard: [D, CP] fp8.
    # x512 puts the tiny xavier-init values in fp8's normal range; the factor
    # cancels exactly in cos = (w.e)/(|w||e|).
    wt_bf = np.zeros((NCORES, D, CP), dtype=f8)
    wr = (w.reshape(NCORES, CS, D) * 512.0).astype(f8)
    for i in range(NCORES):
        wt_bf[i, :, :CS] = wr[i].T
    tw = w[lab]  # [B, D] gathered target rows (f32)
    idm = np.eye(128, dtype=np.float32)

    in_maps = []
    for i in range(NCORES):
        in_maps.append({
            "wt": np.ascontiguousarray(wt_bf[i]),
            "eT": eT_bf,
            "e": emb.astype(f8).astype(np.float32),
            "tw": np.ascontiguousarray(tw[i * 128:(i + 1) * 128]),
            "es": np.ascontiguousarray(emb[i * 128:(i + 1) * 128]),
            "idm": idm,
        })
    return in_maps


def kernel(embeddings, labels, weight):
    global _cached_nc
    in_maps = _prep_inputs(embeddings, labels, weight)
    if _cached_nc is None:
        _cached_nc = _build()
    from concourse.bass_utils import run_bass_kernel_spmd

    r = run_bass_kernel_spmd(_cached_nc, in_maps, core_ids=list(range(NCORES)))
    return np.asarray(r.results[0]["out"][0, 0], dtype=np.float32)


if __name__ == "__main__":
    rng = np.random.default_rng(0)
    emb = rng.normal(size=(B, D)).astype(np.float32)
    lab = rng.integers(0, C, size=(B,)).astype(np.int64)
    lim = float(np.sqrt(6.0 / (C + D)))
    w = rng.uniform(-lim, lim, size=(C, D)).astype(np.float32)
    print(kernel(emb, lab, w))


# revision 31
# speedup vs baseline: 1.0008x; 1.0008x over previous
"""ArcFace loss (B=1024, D=256, C=50000) distributed over 8 TRN2 NeuronCores.

Strategy: shard the class dimension (6250 classes/core, zero-padded to 6272).
Host passes the weight shard pre-transposed ([D, CP], bf16, zero-padded) plus
raw-transposed embeddings.  Per core, on device:
  - |w_c| via TensorE self-matmul (gram diagonal, extracted with an identity
    mask multiply + free-axis reduce on VectorE),
  - all 1/sqrt computed as exp(-0.5*ln(x)) on ScalarE so the whole kernel uses
    a single activation table set (Ln/Exp/Square) - no table-swap stalls,
  - inv-norm row broadcast across partitions (GpSimd) and applied to wT with
    one VectorE multiply (bf16 2x mode),
  - cos = eT.T @ wT_norm on TensorE (bf16), exp on ScalarE with per-partition
    scale 30/|e_b| (folding the embedding norm into the activation scale) and
    free-axis accumulation producing the partial sum-exp,
  - the margin (target-class) term on a [128]-row slice per core from
    host-gathered target weight rows, using
    cos(theta+m) = cos*cos(m) - sqrt(1-cos^2)*sin(m); computed up front so its
    AllGather overlaps the main loop,
  - AllReduce(partial sum-exp) at the end, then log-sum-exp -> mean on device.
The c dimension is processed in groups (4+8+12+12+13 tiles of 128) so the
norm/scale pipeline of group q overlaps the matmul/exp of group q-1, with a
small first group to minimize the serial prefix.
"""

import os
import sys

sys.path.insert(0, "/opt/trn_rl_repo")

import numpy as np
import ml_dtypes

B, D, C = 1024, 256, 50000
NCORES = 8
CS = C // NCORES          # 6250 classes per core
CT = 49                   # 128-class tiles per core
CP = CT * 128             # 6272 (padded)
PADS = CP - CS            # 22 zero-pad classes per core
SCALE = 30.0
MARGIN = 0.5
COSM = float(np.cos(MARGIN))
SINM = float(np.sin(MARGIN))
EPS = 1e-7

# c-tile pipeline groups (in 128-class tiles): small first group -> short prefix
QT = [(0, 5), (5, 8), (13, 12), (25, 12), (37, 12)]
# main-loop c chunks, aligned to group boundaries, <=1536 (3 PSUM banks)
CCH = [(0, 640), (640, 1024), (1664, 1536), (3200, 1536), (4736, 1536)]

_cached_nc = None


def _build(variant="full", niter=1):
    # variant: comma-set of stage-skip flags for benchmarking attribution.
    #   nocc    - skip collectives (use local data instead)
    #   nomain  - skip main matmul+exp loop
    #   nonorm  - skip w-norm pipeline (use raw wT in main loop)
    #   nobcast - replace partition_broadcast with a memset
    vset = set(variant.split(",")) if variant else set()
    from concourse import bacc, tile, mybir

    # Force every ScalarE activation into the one table set that holds all the
    # functions this kernel uses (Ln/Exp/Square/Copy) so the whole NEFF does a
    # single ACT_TABLE_LOAD.  The chooser picks the first set containing each
    # func; hiding these funcs from the other sets (positions preserved, so
    # emitted act_func_set_ids stay valid) redirects it to the combined set.
    import concourse.bacc as _bacc_mod
    from concourse import hw_specs as _hw_specs
    _KEEP = "natural_log_exp_and_others"
    _HIDE = {"Exp", "Ln", "Square", "Copy"}
    if not getattr(_bacc_mod, "_act_tables_patched", False):
        _orig_gat = _hw_specs.get_activation_tables

        def _patched_gat(arch, *a, **kw):
            tabs = _orig_gat(arch, *a, **kw)
            keep = tabs.get(_KEEP)
            if not keep or not _HIDE.issubset({f.name for f in keep}):
                return tabs  # unexpected table layout: leave untouched
            return {
                name: (funcs if name == _KEEP
                       else {f for f in funcs if f.name not in _HIDE})
                for name, funcs in tabs.items()
            }

        try:
            _bacc_mod.get_activation_tables = _patched_gat
            _bacc_mod._act_tables_patched = True
        except Exception:
            pass

    f32 = mybir.dt.float32
    bf16 = mybir.dt.bfloat16
    fp8 = mybir.dt.float8e4
    ALU = mybir.AluOpType
    ACT = mybir.ActivationFunctionType
    AX = mybir.AxisListType

    nc = bacc.Bacc("TRN2", target_bir_lowering=False, debug=False,
                   num_devices=NCORES)

    wtd = nc.dram_tensor("wt", [D, CP], fp8, kind="ExternalInput")
    eTd = nc.dram_tensor("eT", [D, B], fp8, kind="ExternalInput")
    ed = nc.dram_tensor("e", [B, D], f32, kind="ExternalInput")
    twd = nc.dram_tensor("tw", [128, D], f32, kind="ExternalInput")
    esd = nc.dram_tensor("es", [128, D], f32, kind="ExternalInput")
    idd = nc.dram_tensor("idm", [128, 128], f32, kind="ExternalInput")
    outd = nc.dram_tensor("out", [1, 1], f32, kind="ExternalOutput")

    with tile.TileContext(nc) as tc:
        with (
            tc.tile_pool(name="sb", bufs=1) as sb,
            tc.tile_pool(name="ps", bufs=2, space="PSUM") as ps,
            tc.tile_pool(name="gps", bufs=2, space="PSUM") as gps,
            tc.tile_pool(name="dr", bufs=1, space="DRAM") as dr,
        ):
            # ---------------- persistent SBUF tensors ----------------
            wTr = sb.tile([128, 2, CP], fp8)      # raw transposed weights
            wTn = sb.tile([128, 2, CP], fp8)      # normalized
            bcast = sb.tile([128, CP], f32)       # broadcast inv |w_c| row
            eTs = sb.tile([128, 2, B], fp8)
            e_nat = sb.tile([128, 8, D], f32)
            tw_s = sb.tile([128, D], f32)
            es_s = sb.tile([128, D], f32)
            idm = sb.tile([128, 128], f32)
            wt_ap = wtd.ap().rearrange("(k p) c -> p k c", p=128)

            sq_scr = sb.tile([128, 8 * D], f32)   # elementwise-square scratch
            eps_t = sb.tile([128, 1], f32)        # tiny Ln bias: keeps ln(0) finite
            ln30_t = sb.tile([128, 1], f32)       # ln(30) bias for 30/sqrt(x)
            padc_t = sb.tile([128, 1], f32)       # -(pad count) Ln bias
            esq = sb.tile([128, 8], f32)
            lesq = sb.tile([128, 8], f32)
            inv_e30 = sb.tile([128, 8], f32)
            wsq = sb.tile([128, CT], f32)
            lwsq = sb.tile([128, CT], f32)
            inv_wn = sb.tile([128, CT], f32)
            dscr = sb.tile([128, 13, 128], f32)
            rtmp = dr.tile([CP], f32)
            rtmp_pt = rtmp[:].rearrange("(t p) -> p t", p=128)
            rtmp_row = rtmp[:].rearrange("(o c) -> o c", o=1)
            NCC = len(CCH)
            sacc = sb.tile([128, 8 * NCC], f32)
            escr = sb.tile([128, 1536], bf16)
            Sp = sb.tile([128, 8], f32)
            # target-path tiles
            tesq = sb.tile([128, 1], f32)
            tdot = sb.tile([128, 1], f32)
            twsq = sb.tile([128, 1], f32)
            tln = sb.tile([128, 2], f32)
            tinv = sb.tile([128, 2], f32)
            prodd = sb.tile([128, D], f32)
            prod2 = sb.tile([128, D], f32)
            prod3 = sb.tile([128, D], f32)
            ct0 = sb.tile([128, 1], f32)
            ctc = sb.tile([128, 1], f32)
            cos2 = sb.tile([128, 1], f32)
            omc = sb.tile([128, 1], f32)
            lnomc = sb.tile([128, 1], f32)
            sin_t = sb.tile([128, 1], f32)
            ca = sb.tile([128, 1], f32)
            cb = sb.tile([128, 1], f32)
            cosm = sb.tile([128, 1], f32)
            pair = sb.tile([128, 2], f32)
            expts = sb.tile([128, 2], f32)
            ctpay = sb.tile([128, 2], f32)
            # collective buffers
            s_in = dr.tile([128, 8], f32)
            s_out = dr.tile([128, 8], f32)
            c_in = dr.tile([128, 2], f32)
            c_out = dr.tile([B, 2], f32)
            S_ar = sb.tile([128, 8], f32)
            ctg = sb.tile([128, 8, 2], f32)
            S1 = sb.tile([128, 8], f32)
            ctg8 = sb.tile([128, 8], f32)
            S2 = sb.tile([128, 8], f32)
            lse = sb.tile([128, 8], f32)
            nll = sb.tile([128, 8], f32)
            rsum = sb.tile([128, 1], f32)
            ones = sb.tile([128, 1], f32)
            res = sb.tile([1, 1], f32)
            wmm = wTr if "nonorm" in vset else wTn
            grp = [list(range(NCORES))]

            for _it in range(niter):
                # ---- input DMAs: first c-group's weights first (critical path)
                (g0, gn) = QT[0]
                nc.sync.dma_start(out=wTr[:, :, g0 * 128:(g0 + gn) * 128],
                                  in_=wt_ap[:, :, g0 * 128:(g0 + gn) * 128])
                nc.sync.dma_start(out=idm[:], in_=idd.ap())
                nc.sync.dma_start(out=e_nat[:], in_=ed.ap().rearrange("(t p) d -> p t d", p=128))
                nc.sync.dma_start(out=eTs[:], in_=eTd.ap().rearrange("(k p) b -> p k b", p=128))
                nc.sync.dma_start(out=tw_s[:], in_=twd.ap())
                nc.sync.dma_start(out=es_s[:], in_=esd.ap())
                for (t0, nt) in QT[1:]:
                    nc.sync.dma_start(out=wTr[:, :, t0 * 128:(t0 + nt) * 128],
                                      in_=wt_ap[:, :, t0 * 128:(t0 + nt) * 128])

                nc.vector.memset(eps_t[:], 1e-20)
                nc.vector.memset(ln30_t[:], float(np.log(SCALE)))
                nc.vector.memset(padc_t[:], -float(PADS * NCORES))
                nc.vector.memset(ones[:], 1.0 / B)

                # ---- weight norms (gram diag) + normalize, per c-group.
                # Group 0 first (it gates the first main matmul); the e-norm
                # and target paths slot in behind it, then the later groups.
                def _norm_group(t0, nt):
                    for t in range(t0, t0 + nt):
                        g = gps.tile([128, 128], f32, name=f"g{t}", tag="g")
                        nc.tensor.matmul(
                            g[:], lhsT=wTr[:, :, t * 128:(t + 1) * 128],
                            rhs=wTr[:, :, t * 128:(t + 1) * 128],
                            perf_mode=mybir.MatmulPerfMode.DoubleRow,
                            start=True, stop=True)
                        nc.vector.tensor_mul(dscr[:, t - t0], g[:], idm[:])
                    nc.vector.tensor_reduce(out=wsq[:, t0:t0 + nt],
                                            in_=dscr[:, :nt], axis=AX.X, op=ALU.add)
                    nc.scalar.activation(lwsq[:, t0:t0 + nt], wsq[:, t0:t0 + nt],
                                         ACT.Ln, bias=eps_t[:])
                    nc.scalar.activation(inv_wn[:, t0:t0 + nt], lwsq[:, t0:t0 + nt],
                                         ACT.Exp, scale=-0.5)
                    # reorient to a row in DRAM, then DMA the row back
                    # broadcast across all 128 partitions
                    nc.sync.dma_start(out=rtmp_pt[:, t0:t0 + nt],
                                      in_=inv_wn[:, t0:t0 + nt])
                    nc.sync.dma_start(
                        out=bcast[:, t0 * 128:(t0 + nt) * 128],
                        in_=rtmp_row[:, t0 * 128:(t0 + nt) * 128].broadcast_to(
                            (128, nt * 128)))
                    for k in range(2):
                        nc.vector.tensor_mul(wTn[:, k, t0 * 128:(t0 + nt) * 128],
                                             wTr[:, k, t0 * 128:(t0 + nt) * 128],
                                             bcast[:, t0 * 128:(t0 + nt) * 128])

                ngroups = QT if "nonorm" not in vset else []
                if ngroups:
                    _norm_group(*ngroups[0])
                # ---- embedding norms: 30/|e_b| = exp(-0.5 ln(esq) + ln 30)
                for t in range(8):
                    nc.scalar.activation(sq_scr[:, :D], e_nat[:, t], ACT.Square,
                                         accum_out=esq[:, t:t + 1])
                nc.scalar.activation(lesq[:], esq[:], ACT.Ln, bias=eps_t[:])
                nc.scalar.activation(inv_e30[:], lesq[:], ACT.Exp, scale=-0.5,
                                     bias=ln30_t[:])

                # ---- target/margin path first: its AllGather overlaps the rest
                nc.gpsimd.tensor_mul(prodd[:], es_s[:], es_s[:])
                nc.vector.tensor_reduce(out=tesq[:], in_=prodd[:], axis=AX.X, op=ALU.add)
                nc.gpsimd.tensor_mul(prod2[:], tw_s[:], tw_s[:])
                nc.vector.tensor_reduce(out=twsq[:], in_=prod2[:], axis=AX.X, op=ALU.add)
                nc.gpsimd.tensor_mul(prod3[:], es_s[:], tw_s[:])
                nc.vector.tensor_reduce(out=tdot[:], in_=prod3[:], axis=AX.X, op=ALU.add)
                # 1/sqrt via exp(-ln/2): single activation table set
                nc.vector.tensor_copy(pair[:, 0:1], tesq[:])
                nc.vector.tensor_copy(pair[:, 1:2], twsq[:])
                nc.scalar.activation(tln[:], pair[:], ACT.Ln, bias=eps_t[:])
                nc.scalar.activation(tinv[:], tln[:], ACT.Exp, scale=-0.5)
                nc.vector.tensor_mul(ct0[:], tdot[:], tinv[:, 0:1])
                nc.vector.tensor_mul(ctc[:], ct0[:], tinv[:, 1:2])
                nc.vector.tensor_scalar_min(ctc[:], ctc[:], 1.0 - EPS)
                nc.vector.tensor_scalar_max(ctc[:], ctc[:], -1.0 + EPS)
                nc.scalar.activation(cos2[:], ctc[:], ACT.Square)
                nc.vector.tensor_scalar(out=omc[:], in0=cos2[:], scalar1=-1.0,
                                        scalar2=1.0, op0=ALU.mult, op1=ALU.add)
                # sqrt(x) = exp(+ln/2)
                nc.scalar.activation(lnomc[:], omc[:], ACT.Ln, bias=eps_t[:])
                nc.scalar.activation(sin_t[:], lnomc[:], ACT.Exp, scale=0.5)
                nc.vector.tensor_scalar_mul(ca[:], ctc[:], COSM)
                nc.vector.tensor_scalar_mul(cb[:], sin_t[:], SINM)
                nc.vector.tensor_sub(cosm[:], ca[:], cb[:])
                nc.vector.tensor_copy(pair[:, 0:1], ctc[:])
                nc.vector.tensor_copy(pair[:, 1:2], cosm[:])
                nc.scalar.activation(expts[:], pair[:], ACT.Exp, scale=SCALE)
                nc.vector.tensor_sub(ctpay[:, 0:1], expts[:, 1:2], expts[:, 0:1])
                nc.vector.tensor_scalar_mul(ctpay[:, 1:2], cosm[:], SCALE)
                nc.sync.dma_start(out=c_in[:], in_=ctpay[:])
                if "nocc" not in vset:
                    nc.gpsimd.collective_compute(
                        "AllGather", ALU.bypass, replica_groups=grp,
                        ins=[c_in.opt()], outs=[c_out.opt()])
                    nc.sync.dma_start(
                        out=ctg[:], in_=c_out[:].rearrange("(t p) r -> p t r", p=128))
                else:
                    nc.vector.memset(ctg[:], 0.5)

                for (t0, nt) in ngroups[1:]:
                    _norm_group(t0, nt)

                # ---- main matmul + exp + partial sumexp (c-major) ----
                for ci, (off, cs) in enumerate(CCH if "nomain" not in vset else []):
                    for bt in range(8):
                        pt = ps.tile([128, 1536], f32, name=f"pt{ci}_{bt}", tag="pt")
                        for j in range((cs + 511) // 512):
                            n0 = j * 512
                            n1 = min(cs, n0 + 512)
                            nc.tensor.matmul(
                                pt[:, n0:n1],
                                lhsT=eTs[:, :, bt * 128:(bt + 1) * 128],
                                rhs=wmm[:, :, off + n0:off + n1],
                                perf_mode=mybir.MatmulPerfMode.DoubleRow,
                                start=True, stop=True)
                        nc.scalar.activation(
                            escr[:, :cs], pt[:, :cs], ACT.Exp,
                            scale=inv_e30[:, bt:bt + 1],
                            accum_out=sacc[:, bt * NCC + ci:bt * NCC + ci + 1])

                if "nomain" in vset:
                    nc.vector.memset(sacc[:], 1.0)
                nc.vector.tensor_reduce(
                    out=Sp[:], in_=sacc[:].rearrange("p (t c) -> p t c", c=NCC),
                    axis=AX.X, op=ALU.add)
                # pre-add corrections/NCORES before the AllReduce (exact: /8 is
                # a power of two) so no add remains on the post-collective path
                nc.vector.tensor_scalar(out=ctg8[:], in0=ctg[:, :, 0],
                                        scalar1=1.0 / NCORES, scalar2=None,
                                        op0=ALU.mult)
                nc.vector.tensor_add(Sp[:], Sp[:], ctg8[:])
                nc.sync.dma_start(out=s_in[:], in_=Sp[:])
                if "nocc" not in vset:
                    nc.gpsimd.collective_compute(
                        "AllReduce", ALU.add, replica_groups=grp,
                        ins=[s_in.opt()], outs=[s_out.opt()])
                    nc.sync.dma_start(out=S_ar[:], in_=s_out[:])
                else:
                    nc.sync.dma_start(out=S_ar[:], in_=s_in[:])

                # Ln(S - pads): the zero-pad classes contribute exp(0)=1 each
                nc.scalar.activation(lse[:], S_ar[:], ACT.Ln, bias=padc_t[:])
                nc.vector.tensor_sub(nll[:], lse[:], ctg[:, :, 1])
                nc.vector.tensor_reduce(out=rsum[:], in_=nll[:], axis=AX.X, op=ALU.add)
                psf = gps.tile([128, 128], f32, name="psf", tag="g")
                nc.tensor.matmul(psf[0:1, 0:1], lhsT=ones[:], rhs=rsum[:],
                                 start=True, stop=True)
                nc.scalar.copy(res[:], psf[0:1, 0:1])
                nc.sync.dma_start(out=outd.ap(), in_=res[:])

    nc.compile()
    return nc


def _prep_inputs(embeddings, labels, weight):
    emb = np.ascontiguousarray(np.asarray(embeddings), dtype=np.float32)
    lab = np.asarray(labels).astype(np.int64)
    w = np.ascontiguousarray(np.asarray(weight), dtype=np.float32)

    f8 = ml_dtypes.float8_e4m3
    eT_bf = np.ascontiguousarray(emb.T).astype(f8)
    # per-core transposed, zero-padded weight shard: [D, CP] fp8.
    # x512 puts the tiny xavier-init values in fp8's normal range; the factor
    # cancels exactly in cos = (w.e)/(|w||e|).
    wt_bf = np.zeros((NCORES, D, CP), dtype=f8)
    wr = (w.reshape(NCORES, CS, D) * 512.0).astype(f8)
    for i in range(NCORES):
        wt_bf[i, :, :CS] = wr[i].T
    tw = w[lab]  # [B, D] gathered target rows (f32)
    idm = np.eye(128, dtype=np.float32)

    in_maps = []
    for i in range(NCORES):
        in_maps.append({
            "wt": np.ascontiguousarray(wt_bf[i]),
            "eT": eT_bf,
            "e": emb.astype(f8).astype(np.float32),
            "tw": np.ascontiguousarray(tw[i * 128:(i + 1) * 128]),
            "es": np.ascontiguousarray(emb[i * 128:(i + 1) * 128]),
            "idm": idm,
        })
    return in_maps


def kernel(embeddings, labels, weight):
    global _cached_nc
    in_maps = _prep_inputs(embeddings, labels, weight)
    if _cached_nc is None:
        _cached_nc = _build()
    from concourse.bass_utils import run_bass_kernel_spmd

    r = run_bass_kernel_spmd(_cached_nc, in_maps, core_ids=list(range(NCORES)))
    return np.asarray(r.results[0]["out"][0, 0], dtype=np.float32)


if __name__ == "__main__":
    rng = np.random.default_rng(0)
    emb = rng.normal(size=(B, D)).astype(np.float32)
    lab = rng.integers(0, C, size=(B,)).astype(np.int64)
    lim = float(np.sqrt(6.0 / (C + D)))
    w = rng.uniform(-lim, lim, size=(C, D)).astype(np.float32)
    print(kernel(emb, lab, w))


# revision 32
# speedup vs baseline: 1.0809x; 1.0800x over previous
"""ArcFace loss (B=1024, D=256, C=50000) distributed over 8 TRN2 NeuronCores.

Strategy: shard the class dimension (6250 classes/core, zero-padded to 6272).
Host passes the weight shard pre-transposed ([D, CP], bf16, zero-padded) plus
raw-transposed embeddings.  Per core, on device:
  - |w_c| via TensorE self-matmul (gram diagonal, extracted with an identity
    mask multiply + free-axis reduce on VectorE),
  - all 1/sqrt computed as exp(-0.5*ln(x)) on ScalarE so the whole kernel uses
    a single activation table set (Ln/Exp/Square) - no table-swap stalls,
  - inv-norm row broadcast across partitions (GpSimd) and applied to wT with
    one VectorE multiply (bf16 2x mode),
  - cos = eT.T @ wT_norm on TensorE (bf16), exp on ScalarE with per-partition
    scale 30/|e_b| (folding the embedding norm into the activation scale) and
    free-axis accumulation producing the partial sum-exp,
  - the margin (target-class) term on a [128]-row slice per core from
    host-gathered target weight rows, using
    cos(theta+m) = cos*cos(m) - sqrt(1-cos^2)*sin(m); computed up front so its
    AllGather overlaps the main loop,
  - AllReduce(partial sum-exp) at the end, then log-sum-exp -> mean on device.
The c dimension is processed in groups (4+8+12+12+13 tiles of 128) so the
norm/scale pipeline of group q overlaps the matmul/exp of group q-1, with a
small first group to minimize the serial prefix.
"""

import os
import sys

sys.path.insert(0, "/opt/trn_rl_repo")

import numpy as np
import ml_dtypes

B, D, C = 1024, 256, 50000
NCORES = 8
CS = C // NCORES          # 6250 classes per core
CT = 49                   # 128-class tiles per core
CP = CT * 128             # 6272 (padded)
PADS = CP - CS            # 22 zero-pad classes per core
SCALE = 30.0
MARGIN = 0.5
COSM = float(np.cos(MARGIN))
SINM = float(np.sin(MARGIN))
EPS = 1e-7

# c-tile pipeline groups (in 128-class tiles): small first group -> short prefix
QT = [(0, 5), (5, 8), (13, 12), (25, 12), (37, 12)]
# main-loop c chunks, aligned to group boundaries, <=1536 (3 PSUM banks)
CCH = [(0, 640), (640, 1024), (1664, 1536), (3200, 1536), (4736, 1536)]

_cached_nc = None


def _build(variant="full", niter=1):
    # variant: comma-set of stage-skip flags for benchmarking attribution.
    #   nocc    - skip collectives (use local data instead)
    #   nomain  - skip main matmul+exp loop
    #   nonorm  - skip w-norm pipeline (use raw wT in main loop)
    #   nobcast - replace partition_broadcast with a memset
    vset = set(variant.split(",")) if variant else set()
    from concourse import bacc, tile, mybir

    # Force every ScalarE activation into the one table set that holds all the
    # functions this kernel uses (Ln/Exp/Square/Copy) so the whole NEFF does a
    # single ACT_TABLE_LOAD.  The chooser picks the first set containing each
    # func; hiding these funcs from the other sets (positions preserved, so
    # emitted act_func_set_ids stay valid) redirects it to the combined set.
    import concourse.bacc as _bacc_mod
    from concourse import hw_specs as _hw_specs
    _KEEP = "natural_log_exp_and_others"
    _HIDE = {"Exp", "Ln", "Square", "Copy"}
    if not getattr(_bacc_mod, "_act_tables_patched", False):
        _orig_gat = _hw_specs.get_activation_tables

        def _patched_gat(arch, *a, **kw):
            tabs = _orig_gat(arch, *a, **kw)
            keep = tabs.get(_KEEP)
            if not keep or not _HIDE.issubset({f.name for f in keep}):
                return tabs  # unexpected table layout: leave untouched
            return {
                name: (funcs if name == _KEEP
                       else {f for f in funcs if f.name not in _HIDE})
                for name, funcs in tabs.items()
            }

        try:
            _bacc_mod.get_activation_tables = _patched_gat
            _bacc_mod._act_tables_patched = True
        except Exception:
            pass

    f32 = mybir.dt.float32
    bf16 = mybir.dt.bfloat16
    fp8 = mybir.dt.float8e4
    ALU = mybir.AluOpType
    ACT = mybir.ActivationFunctionType
    AX = mybir.AxisListType

    nc = bacc.Bacc("TRN2", target_bir_lowering=False, debug=False,
                   num_devices=NCORES)

    wtd = nc.dram_tensor("wt", [D, CP], fp8, kind="ExternalInput")
    eTd = nc.dram_tensor("eT", [D, B], fp8, kind="ExternalInput")
    ed = nc.dram_tensor("e", [B, D], f32, kind="ExternalInput")
    twd = nc.dram_tensor("tw", [128, D], f32, kind="ExternalInput")
    esd = nc.dram_tensor("es", [128, D], f32, kind="ExternalInput")
    idd = nc.dram_tensor("idm", [128, 128], f32, kind="ExternalInput")
    outd = nc.dram_tensor("out", [1, 1], f32, kind="ExternalOutput")

    with tile.TileContext(nc) as tc:
        with (
            tc.tile_pool(name="sb", bufs=1) as sb,
            tc.tile_pool(name="ps", bufs=2, space="PSUM") as ps,
            tc.tile_pool(name="gps", bufs=2, space="PSUM") as gps,
            tc.tile_pool(name="dr", bufs=1, space="DRAM") as dr,
        ):
            # ---------------- persistent SBUF tensors ----------------
            wTr = sb.tile([128, 2, CP], fp8)      # raw transposed weights
            wTn = sb.tile([128, 2, CP], fp8)      # normalized
            bcast = sb.tile([128, CP], f32)       # broadcast inv |w_c| row
            eTs = sb.tile([128, 2, B], fp8)
            e_nat = sb.tile([128, 8, D], f32)
            tw_s = sb.tile([128, D], f32)
            es_s = sb.tile([128, D], f32)
            idm = sb.tile([128, 128], f32)
            wt_ap = wtd.ap().rearrange("(k p) c -> p k c", p=128)

            sq_scr = sb.tile([128, 8 * D], f32)   # elementwise-square scratch
            eps_t = sb.tile([128, 1], f32)        # tiny Ln bias: keeps ln(0) finite
            ln30_t = sb.tile([128, 1], f32)       # ln(30) bias for 30/sqrt(x)
            padc_t = sb.tile([128, 1], f32)       # -(pad count) Ln bias
            esq = sb.tile([128, 8], f32)
            lesq = sb.tile([128, 8], f32)
            inv_e30 = sb.tile([128, 8], f32)
            wsq = sb.tile([128, CT], f32)
            lwsq = sb.tile([128, CT], f32)
            inv_wn = sb.tile([128, CT], f32)
            dscr = sb.tile([128, 13, 128], f32)
            rtmp = dr.tile([CP], f32)
            rtmp_pt = rtmp[:].rearrange("(t p) -> p t", p=128)
            rtmp_row = rtmp[:].rearrange("(o c) -> o c", o=1)
            NCC = len(CCH)
            sacc = sb.tile([128, 8 * NCC], f32)
            escr = sb.tile([128, 1536], bf16)
            Sp = sb.tile([128, 8], f32)
            # target-path tiles
            tesq = sb.tile([128, 1], f32)
            tdot = sb.tile([128, 1], f32)
            twsq = sb.tile([128, 1], f32)
            tln = sb.tile([128, 2], f32)
            tinv = sb.tile([128, 2], f32)
            prodd = sb.tile([128, D], f32)
            prod2 = sb.tile([128, D], f32)
            prod3 = sb.tile([128, D], f32)
            ct0 = sb.tile([128, 1], f32)
            ctc = sb.tile([128, 1], f32)
            cos2 = sb.tile([128, 1], f32)
            omc = sb.tile([128, 1], f32)
            lnomc = sb.tile([128, 1], f32)
            sin_t = sb.tile([128, 1], f32)
            ca = sb.tile([128, 1], f32)
            cb = sb.tile([128, 1], f32)
            cosm = sb.tile([128, 1], f32)
            pair = sb.tile([128, 2], f32)
            expts = sb.tile([128, 2], f32)
            ctpay = sb.tile([128, 2], f32)
            # collective buffers
            s_in = dr.tile([128, 8], f32)
            s_out = dr.tile([B, 8], f32)
            S_g = sb.tile([128, 8, 8], f32)
            c_in = dr.tile([128, 2], f32)
            c_out = dr.tile([B, 2], f32)
            S_ar = sb.tile([128, 8], f32)
            ctg = sb.tile([128, 8, 2], f32)
            S1 = sb.tile([128, 8], f32)
            ctg8 = sb.tile([128, 8], f32)
            S2 = sb.tile([128, 8], f32)
            lse = sb.tile([128, 8], f32)
            nll = sb.tile([128, 8], f32)
            rsum = sb.tile([128, 1], f32)
            ones = sb.tile([128, 1], f32)
            res = sb.tile([1, 1], f32)
            wmm = wTr if "nonorm" in vset else wTn
            grp = [list(range(NCORES))]

            for _it in range(niter):
                # ---- input DMAs: first c-group's weights first (critical path)
                (g0, gn) = QT[0]
                nc.sync.dma_start(out=wTr[:, :, g0 * 128:(g0 + gn) * 128],
                                  in_=wt_ap[:, :, g0 * 128:(g0 + gn) * 128])
                nc.sync.dma_start(out=idm[:], in_=idd.ap())
                nc.sync.dma_start(out=e_nat[:], in_=ed.ap().rearrange("(t p) d -> p t d", p=128))
                nc.sync.dma_start(out=eTs[:], in_=eTd.ap().rearrange("(k p) b -> p k b", p=128))
                nc.sync.dma_start(out=tw_s[:], in_=twd.ap())
                nc.sync.dma_start(out=es_s[:], in_=esd.ap())
                for (t0, nt) in QT[1:]:
                    nc.sync.dma_start(out=wTr[:, :, t0 * 128:(t0 + nt) * 128],
                                      in_=wt_ap[:, :, t0 * 128:(t0 + nt) * 128])

                nc.vector.memset(eps_t[:], 1e-20)
                nc.vector.memset(ln30_t[:], float(np.log(SCALE)))
                nc.vector.memset(padc_t[:], -float(PADS * NCORES))
                nc.vector.memset(ones[:], 1.0 / B)

                # ---- weight norms (gram diag) + normalize, per c-group.
                # Group 0 first (it gates the first main matmul); the e-norm
                # and target paths slot in behind it, then the later groups.
                def _norm_group(t0, nt):
                    for t in range(t0, t0 + nt):
                        g = gps.tile([128, 128], f32, name=f"g{t}", tag="g")
                        nc.tensor.matmul(
                            g[:], lhsT=wTr[:, :, t * 128:(t + 1) * 128],
                            rhs=wTr[:, :, t * 128:(t + 1) * 128],
                            perf_mode=mybir.MatmulPerfMode.DoubleRow,
                            start=True, stop=True)
                        nc.vector.tensor_mul(dscr[:, t - t0], g[:], idm[:])
                    nc.vector.tensor_reduce(out=wsq[:, t0:t0 + nt],
                                            in_=dscr[:, :nt], axis=AX.X, op=ALU.add)
                    nc.scalar.activation(lwsq[:, t0:t0 + nt], wsq[:, t0:t0 + nt],
                                         ACT.Ln, bias=eps_t[:])
                    nc.scalar.activation(inv_wn[:, t0:t0 + nt], lwsq[:, t0:t0 + nt],
                                         ACT.Exp, scale=-0.5)
                    # reorient to a row in DRAM, then DMA the row back
                    # broadcast across all 128 partitions
                    nc.sync.dma_start(out=rtmp_pt[:, t0:t0 + nt],
                                      in_=inv_wn[:, t0:t0 + nt])
                    nc.sync.dma_start(
                        out=bcast[:, t0 * 128:(t0 + nt) * 128],
                        in_=rtmp_row[:, t0 * 128:(t0 + nt) * 128].broadcast_to(
                            (128, nt * 128)))
                    for k in range(2):
                        nc.vector.tensor_mul(wTn[:, k, t0 * 128:(t0 + nt) * 128],
                                             wTr[:, k, t0 * 128:(t0 + nt) * 128],
                                             bcast[:, t0 * 128:(t0 + nt) * 128])

                ngroups = QT if "nonorm" not in vset else []
                if ngroups:
                    _norm_group(*ngroups[0])
                # ---- embedding norms: 30/|e_b| = exp(-0.5 ln(esq) + ln 30)
                for t in range(8):
                    nc.scalar.activation(sq_scr[:, :D], e_nat[:, t], ACT.Square,
                                         accum_out=esq[:, t:t + 1])
                nc.scalar.activation(lesq[:], esq[:], ACT.Ln, bias=eps_t[:])
                nc.scalar.activation(inv_e30[:], lesq[:], ACT.Exp, scale=-0.5,
                                     bias=ln30_t[:])

                # ---- target/margin path first: its AllGather overlaps the rest
                nc.gpsimd.tensor_mul(prodd[:], es_s[:], es_s[:])
                nc.vector.tensor_reduce(out=tesq[:], in_=prodd[:], axis=AX.X, op=ALU.add)
                nc.gpsimd.tensor_mul(prod2[:], tw_s[:], tw_s[:])
                nc.vector.tensor_reduce(out=twsq[:], in_=prod2[:], axis=AX.X, op=ALU.add)
                nc.gpsimd.tensor_mul(prod3[:], es_s[:], tw_s[:])
                nc.vector.tensor_reduce(out=tdot[:], in_=prod3[:], axis=AX.X, op=ALU.add)
                # 1/sqrt via exp(-ln/2): single activation table set
                nc.vector.tensor_copy(pair[:, 0:1], tesq[:])
                nc.vector.tensor_copy(pair[:, 1:2], twsq[:])
                nc.scalar.activation(tln[:], pair[:], ACT.Ln, bias=eps_t[:])
                nc.scalar.activation(tinv[:], tln[:], ACT.Exp, scale=-0.5)
                nc.vector.tensor_mul(ct0[:], tdot[:], tinv[:, 0:1])
                nc.vector.tensor_mul(ctc[:], ct0[:], tinv[:, 1:2])
                nc.vector.tensor_scalar_min(ctc[:], ctc[:], 1.0 - EPS)
                nc.vector.tensor_scalar_max(ctc[:], ctc[:], -1.0 + EPS)
                nc.scalar.activation(cos2[:], ctc[:], ACT.Square)
                nc.vector.tensor_scalar(out=omc[:], in0=cos2[:], scalar1=-1.0,
                                        scalar2=1.0, op0=ALU.mult, op1=ALU.add)
                # sqrt(x) = exp(+ln/2)
                nc.scalar.activation(lnomc[:], omc[:], ACT.Ln, bias=eps_t[:])
                nc.scalar.activation(sin_t[:], lnomc[:], ACT.Exp, scale=0.5)
                nc.vector.tensor_scalar_mul(ca[:], ctc[:], COSM)
                nc.vector.tensor_scalar_mul(cb[:], sin_t[:], SINM)
                nc.vector.tensor_sub(cosm[:], ca[:], cb[:])
                nc.vector.tensor_copy(pair[:, 0:1], ctc[:])
                nc.vector.tensor_copy(pair[:, 1:2], cosm[:])
                nc.scalar.activation(expts[:], pair[:], ACT.Exp, scale=SCALE)
                nc.vector.tensor_sub(ctpay[:, 0:1], expts[:, 1:2], expts[:, 0:1])
                nc.vector.tensor_scalar_mul(ctpay[:, 1:2], cosm[:], SCALE)
                nc.sync.dma_start(out=c_in[:], in_=ctpay[:])
                if "nocc" not in vset:
                    nc.gpsimd.collective_compute(
                        "AllGather", ALU.bypass, replica_groups=grp,
                        ins=[c_in.opt()], outs=[c_out.opt()])
                    nc.sync.dma_start(
                        out=ctg[:], in_=c_out[:].rearrange("(t p) r -> p t r", p=128))
                else:
                    nc.vector.memset(ctg[:], 0.5)

                for (t0, nt) in ngroups[1:]:
                    _norm_group(t0, nt)

                # ---- main matmul + exp + partial sumexp (c-major) ----
                for ci, (off, cs) in enumerate(CCH if "nomain" not in vset else []):
                    for bt in range(8):
                        pt = ps.tile([128, 1536], f32, name=f"pt{ci}_{bt}", tag="pt")
                        for j in range((cs + 511) // 512):
                            n0 = j * 512
                            n1 = min(cs, n0 + 512)
                            nc.tensor.matmul(
                                pt[:, n0:n1],
                                lhsT=eTs[:, :, bt * 128:(bt + 1) * 128],
                                rhs=wmm[:, :, off + n0:off + n1],
                                perf_mode=mybir.MatmulPerfMode.DoubleRow,
                                start=True, stop=True)
                        nc.scalar.activation(
                            escr[:, :cs], pt[:, :cs], ACT.Exp,
                            scale=inv_e30[:, bt:bt + 1],
                            accum_out=sacc[:, bt * NCC + ci:bt * NCC + ci + 1])

                if "nomain" in vset:
                    nc.vector.memset(sacc[:], 1.0)
                nc.vector.tensor_reduce(
                    out=Sp[:], in_=sacc[:].rearrange("p (t c) -> p t c", c=NCC),
                    axis=AX.X, op=ALU.add)
                # pre-add corrections/NCORES before the AllReduce (exact: /8 is
                # a power of two) so no add remains on the post-collective path
                nc.vector.tensor_scalar(out=ctg8[:], in0=ctg[:, :, 0],
                                        scalar1=1.0 / NCORES, scalar2=None,
                                        op0=ALU.mult)
                nc.vector.tensor_add(Sp[:], Sp[:], ctg8[:])
                nc.sync.dma_start(out=s_in[:], in_=Sp[:])
                # AllGather + local sum instead of AllReduce: an AllReduce is
                # internally reduce-scatter + all-gather (~1.9x the latency);
                # for this tiny latency-bound payload gathering the 8 partial
                # vectors and summing them on VectorE is faster.
                if "nocc" not in vset:
                    nc.gpsimd.collective_compute(
                        "AllGather", ALU.bypass, replica_groups=grp,
                        ins=[s_in.opt()], outs=[s_out.opt()])
                    nc.sync.dma_start(
                        out=S_g[:], in_=s_out[:].rearrange("(c p) b -> p b c", p=128))
                else:
                    nc.sync.dma_start(
                        out=S_g[:],
                        in_=s_in[:].rearrange("p b -> p b 1").broadcast_to((128, 8, 8)))
                nc.vector.tensor_reduce(out=S_ar[:], in_=S_g[:], axis=AX.X,
                                        op=ALU.add)

                # Ln(S - pads): the zero-pad classes contribute exp(0)=1 each
                nc.scalar.activation(lse[:], S_ar[:], ACT.Ln, bias=padc_t[:])
                nc.vector.tensor_sub(nll[:], lse[:], ctg[:, :, 1])
                nc.vector.tensor_reduce(out=rsum[:], in_=nll[:], axis=AX.X, op=ALU.add)
                psf = gps.tile([128, 128], f32, name="psf", tag="g")
                nc.tensor.matmul(psf[0:1, 0:1], lhsT=ones[:], rhs=rsum[:],
                                 start=True, stop=True)
                nc.scalar.copy(res[:], psf[0:1, 0:1])
                nc.sync.dma_start(out=outd.ap(), in_=res[:])

    nc.compile()
    return nc


def _prep_inputs(embeddings, labels, weight):
    emb = np.ascontiguousarray(np.asarray(embeddings), dtype=np.float32)
    lab = np.asarray(labels).astype(np.int64)
    w = np.ascontiguousarray(np.asarray(weight), dtype=np.float32)

    f8 = ml_dtypes.float8_e4m3
    eT_bf = np.ascontiguousarray(emb.T).astype(f8)
    # per-core transposed, zero-padded weight shard: [D, CP] fp8.
    # x512 puts the tiny xavier-init values in fp8's normal range; the factor
    # cancels exactly in cos = (w.e)/(|w||e|).
    wt_bf = np.zeros((NCORES, D, CP), dtype=f8)
    wr = (w.reshape(NCORES, CS, D) * 512.0).astype(f8)
    for i in range(NCORES):
        wt_bf[i, :, :CS] = wr[i].T
    tw = w[lab]  # [B, D] gathered target rows (f32)
    idm = np.eye(128, dtype=np.float32)

    in_maps = []
    for i in range(NCORES):
        in_maps.append({
            "wt": np.ascontiguousarray(wt_bf[i]),
            "eT": eT_bf,
            "e": emb.astype(f8).astype(np.float32),
            "tw": np.ascontiguousarray(tw[i * 128:(i + 1) * 128]),
            "es": np.ascontiguousarray(emb[i * 128:(i + 1) * 128]),
            "idm": idm,
        })
    return in_maps


def kernel(embeddings, labels, weight):
    global _cached_nc
    in_maps = _prep_inputs(embeddings, labels, weight)
    if _cached_nc is None:
        _cached_nc = _build()
    from concourse.bass_utils import run_bass_kernel_spmd

    r = run_bass_kernel_spmd(_cached_nc, in_maps, core_ids=list(range(NCORES)))
    return np.asarray(r.results[0]["out"][0, 0], dtype=np.float32)


if __name__ == "__main__":
    rng = np.random.default_rng(0)
    emb = rng.normal(size=(B, D)).astype(np.float32)
    lab = rng.integers(0, C, size=(B,)).astype(np.int64)
    lim = float(np.sqrt(6.0 / (C + D)))
    w = rng.uniform(-lim, lim, size=(C, D)).astype(np.float32)
    print(kernel(emb, lab, w))
